# revision 11
# baseline (speedup 1.0000x reference)
"""DSH loss kernel for Trainium2 (8 NeuronCores, Bass/Tile) — v6.

Math (reference):
    U[ind] = u; Y[ind] = y
    raw[b,n]  = ||u_b||^2 - 2 u_b.U_n + ||U_n||^2
    dist      = max(raw, 0)
    match[b,n]= y_b . Y_n          (integer >= 0)
    m         = (match == 0)       (statistically ~never 1)
    loss1 = mean( (1-m)*0.5*dist + m*0.5*relu(M - dist) )
    loss2 = ALPHA * mean(|1 - sign(u)|)

Decomposition (host, fp64, exact):
    2*B*N*loss1 = S_raw + sum_{match=0 pairs} [ relu(M - raw) - raw ]
    S_raw factorizes into colsums -> host. The device's only job is a
    SOUND per-batch-row flag "row b has a match==0 pair on this shard";
    the host recomputes flagged rows exactly.

Device architecture (v6): flipped + fp16 bias-packed + bit-field tests.
    Constraints discovered on this part: PE runs at 1.2 GHz (never
    ramps), so unpacked match matmuls (1 cy/gallery-row) are PE-bound;
    the elementwise PSUM drain is limited to ACT+DVE at 1 elem/lane-cy
    (GpSimd has no PSUM port); mod is not a legal ALU op on the
    tensor_scalar accumulate path.  Solution: pack TWO gallery rows per
    PSUM value with an exponent-pinning bias, entirely with ISA-legal
    ops:
        P = 16384 + 0.5*m_e + 128*m_o        (fp32, EXACT: 16-bit span)
    via one fp16 matmul per tile: moving operand rows hold
    0.5*Y[2j,k] + 128*Y[2j+1,k] in {0, 0.5, 128, 128.5} (fp16-exact)
    plus a constant row pairing ypT row 100 == 1.0 with 16384.0.
    Since P is pinned to [2^14, 2^15), its fp32 bits are
    [sign|exp=141|m_o (7b)|m_e (7b)|0...]: the LOW bf16 half of P is
    +0.0 iff m_e == 0 (else a positive normal 2^(2*m_e-127)), and the
    HIGH half/whole P gives m_o by threshold. Detection per tile:
        O-test (m_o==0):  P < 16448          (threshold)
        E-test (m_e==0):  low-half(P) == 0   (bitcast bf16 view)
    EACH test runs on EITHER engine (greedy balance, separate accum
    tensors per engine to avoid cross-engine false deps):
        O-ACT: relu(16448 - P) sum-accum          (flag col > 7)
        O-DVE: min(P - 16448, 0) min-accum        (flag col < -7)
        E-ACT: relu(1 - 2^126 * L) sum-accum      (flag col > 0.5)
        E-DVE: tensor_reduce min over L           (flag col < 1.17e-38)
    All exact integer/bit tests -- no false negatives possible with
    binary labels; the rare false positive (none on real data) only
    costs host recheck time.

Tiling per core (shard = 12500 gallery rows = 6250 packed cols, padded
to 6272 = 3*2048 + 128): 4 batch-chunk sweeps x 3 big tiles
[128, 2048] fp32 PSUM (4 banks, pool bufs=2 = all 8); the 4 sweeps'
128-col remainders merge into one [128, 512] tail tile processed last.
PE cost: 25088 cy @1.2GHz ~= 21 us, under the ~30 us EW wall. Pad
packed cols hold 128.5 (+16384 bias row) => P = 16384 + 128.5*|y_b|,
never flagged unless y_b == 0 (sound). Gallery DMA is sliced across
the sync + gpsimd queues; ypT goes first so matmul 0 starts early.
"""

import numpy as np
import ml_dtypes

import concourse.bass as bass
import concourse.mybir as mybir
import concourse.tile as tile
from concourse import bacc
from concourse.bass_utils import run_bass_kernel_spmd

# Problem constants (hardcoded per harness contract)
B = 512
BIT = 64
C = 100
N = 100000
N_CORES = 8
N_SH = N // N_CORES          # 12500
M_MARGIN = 2.0 * BIT         # 128.0
ALPHA = 0.1
BIG_TILE = 2048              # packed cols per big PSUM tile (4 banks)
N_CHUNK = 4                  # batch chunks of 128 (PSUM partition dim)
BIAS = 16384.0               # exponent-pinning bias (2^14)
OTHR = 16448.0               # O-test threshold: 16384 + 64
ESCALE = -float(2.0 ** 126)  # E-ACT scale

# modeled per-op engine cost (ns): cost = per_col * w + overhead
_ACT_COL, _ACT_OH = 1.084, 620.0   # incl. read-accumulator
_DVE_COL, _DVE_OH = 1.192, 300.0

FP16 = np.float16
BF16 = ml_dtypes.bfloat16


def _plan_tiles(n_pk_pad: int):
    big = []
    left = n_pk_pad
    while left >= BIG_TILE:
        big.append(BIG_TILE)
        left -= BIG_TILE
    return big, left


def _make_schedule(n_sh: int):
    """tiles + greedy (engine, acc-col) assignment for the O/E op pair."""
    n_pk = n_sh // 2
    n_pk_pad = ((n_pk + 127) // 128) * 128
    big_w, rem = _plan_tiles(n_pk_pad)
    merge_rem = 0 < rem <= 512

    tiles = []
    for j in range(N_CHUNK):
        for i, w in enumerate(big_w):
            tiles.append(("big", j, i * BIG_TILE, w))
        if rem and not merge_rem:
            tiles.append(("rem", j, len(big_w) * BIG_TILE, rem))
    if merge_rem:
        tiles.append(("tail", -1, len(big_w) * BIG_TILE, rem))

    # greedy balance: for each tile, assign O then E to the engine with
    # the smaller accumulated busy time
    t_act = t_dve = 0.0
    na = nv = 0
    ops = []       # per tile: [(test, engine, col), (test, engine, col)]
    for kind, j, c0, w in tiles:
        ew = N_CHUNK * w if kind == "tail" else w
        pair = []
        for test in ("O", "E"):
            ca = _ACT_COL * ew + _ACT_OH
            cv = _DVE_COL * ew + _DVE_OH
            if t_act + ca <= t_dve + cv:
                pair.append((test, "A", na))
                t_act += ca
                na += 1
            else:
                pair.append((test, "D", nv))
                t_dve += cv
                nv += 1
        ops.append(pair)
    return tiles, ops, rem, max(na, 1), max(nv, 1)


def _build_program(n_sh: int):
    fp32 = mybir.dt.float32
    bf16 = mybir.dt.bfloat16
    fp16 = mybir.dt.float16
    nc = bacc.Bacc("TRN2", target_bir_lowering=False)

    assert n_sh % 2 == 0
    n_pk = n_sh // 2
    n_pk_pad = ((n_pk + 127) // 128) * 128
    tiles, ops, rem, na, nv = _make_schedule(n_sh)

    ypT_d = nc.declare_dram_parameter("ypT", [128, B], fp16, isOutput=False)
    Ypk_d = nc.declare_dram_parameter("Ypk", [128, n_pk_pad], fp16, isOutput=False)
    accA_d = nc.declare_dram_parameter("accA", [128, na], fp32, isOutput=True)
    accV_d = nc.declare_dram_parameter("accV", [128, nv], fp32, isOutput=True)

    with tile.TileContext(nc) as tc:
        with (
            tc.tile_pool(name="resident", bufs=1) as resident,
            tc.tile_pool(name="scr", bufs=2) as scrp,
            tc.tile_pool(name="psum", bufs=2, space="PSUM") as psump,
        ):
            yp_sb = resident.tile([128, B], fp16, tag="yp")
            Ypk_sb = resident.tile([128, n_pk_pad], fp16, tag="Ypk")
            accA = resident.tile([128, na], fp32, tag="accA")
            accV = resident.tile([128, nv], fp32, tag="accV")
            bias_o = resident.tile([128, 1], fp32, tag="biaso")
            bias_1 = resident.tile([128, 1], fp32, tag="bias1")

            nc.sync.dma_start(yp_sb[:], ypT_d[:])
            chunks = [1024, 1024, 2048, 2048]
            qs = [nc.gpsimd, nc.sync]
            s = 0
            qi = 0
            while s < n_pk_pad:
                w = min(chunks.pop(0) if chunks else 2048, n_pk_pad - s)
                qs[qi].dma_start(Ypk_sb[:, s : s + w], Ypk_d[:, s : s + w])
                qi ^= 1
                s += w

            nc.vector.memset(bias_o[:], OTHR)
            nc.vector.memset(bias_1[:], 1.0)
            nc.vector.memset(accA[:], 0.0)
            nc.vector.memset(accV[:], 0.0)

            for ti, (kind, j, c0, w) in enumerate(tiles):
                x = psump.tile([128, BIG_TILE], fp32, tag="x")
                if kind == "tail":
                    for q in range(N_CHUNK):
                        nc.tensor.matmul(
                            x[:, q * rem : (q + 1) * rem],
                            lhsT=yp_sb[:, q * 128 : (q + 1) * 128],
                            rhs=Ypk_sb[:, c0 : c0 + rem],
                            start=True, stop=True,
                        )
                    ew = N_CHUNK * rem
                else:
                    for m0 in range(0, w, 512):
                        mw = min(512, w - m0)
                        nc.tensor.matmul(
                            x[:, m0 : m0 + mw],
                            lhsT=yp_sb[:, j * 128 : (j + 1) * 128],
                            rhs=Ypk_sb[:, c0 + m0 : c0 + m0 + mw],
                            start=True, stop=True,
                        )
                    ew = w

                xb = x[:, :ew].bitcast(bf16)
                xlow = xb[:, 0::2]
                for test, eng, col in ops[ti]:
                    if eng == "A":
                        scr = scrp.tile([128, BIG_TILE], bf16,
                                        tag="sA" + test)
                        if test == "O":
                            # relu(16448 - P): >= 14 per m_o==0 pair
                            nc.scalar.activation(
                                scr[:, :ew], x[:, :ew],
                                mybir.ActivationFunctionType.Relu,
                                bias=bias_o[:], scale=-1.0,
                                accum_out=accA[:, col : col + 1],
                            )
                        else:
                            # relu(1 - 2^126*L): 1 iff L == +0 (m_e==0)
                            nc.scalar.activation(
                                scr[:, :ew], xlow,
                                mybir.ActivationFunctionType.Relu,
                                bias=bias_1[:], scale=ESCALE,
                                accum_out=accA[:, col : col + 1],
                            )
                    else:
                        if test == "O":
                            scr = scrp.tile([128, BIG_TILE], bf16, tag="sDO")
                            # min(P - 16448, 0) min-accum: <= -14 iff m_o==0
                            nc.vector.tensor_scalar(
                                scr[:, :ew], x[:, :ew], OTHR, 0.0,
                                mybir.AluOpType.subtract, mybir.AluOpType.min,
                                accum_out=accV[:, col : col + 1],
                            )
                        else:
                            # min over low halves: == +0 iff some m_e==0
                            nc.vector.tensor_reduce(
                                accV[:, col : col + 1], xlow,
                                axis=mybir.AxisListType.X,
                                op=mybir.AluOpType.min,
                            )

            for d, t, n in ((accA_d, accA, na), (accV_d, accV, nv)):
                cut = max(0, n - 2)
                if cut:
                    nc.sync.dma_start(d[:, :cut], t[:, :cut])
                nc.sync.dma_start(d[:, cut:], t[:, cut:])

    nc.finalize()
    return nc, tiles, ops, rem


def _pad_shard(Ypk_shard):
    """Pad a packed shard to the 128-multiple width the program expects.

    Pad cols: rows <C hold 128.5, row C holds the 16384 bias => for any
    batch row, P_pad = 16384 + 128.5*|y_b|: O-test >= 16512.5 and
    E-field = |y_b| -- only an all-zero y_b fires (sound).
    """
    n_pk = Ypk_shard.shape[1]
    n_pk_pad = ((n_pk + 127) // 128) * 128
    if n_pk_pad == n_pk:
        return np.ascontiguousarray(Ypk_shard)
    pad = np.zeros((128, n_pk_pad - n_pk), dtype=FP16)
    pad[:C] = FP16(128.5)
    pad[C] = FP16(BIAS)
    return np.ascontiguousarray(np.concatenate([Ypk_shard, pad], axis=1))


def _host_flags(accA, accV, tiles, ops):
    """acc tensors -> set of suspicious batch rows."""
    bad = set()
    for ti, (kind, j, c0, w) in enumerate(tiles):
        ps = []
        for test, eng, col in ops[ti]:
            if eng == "A":
                c = accA[:, col]
                ps.append(c > (7.0 if test == "O" else 0.5))
            else:
                c = accV[:, col]
                ps.append(c < (-7.0 if test == "O" else 1.17e-38))
        hit = np.nonzero(ps[0] | ps[1])[0]
        for p in hit:
            if kind == "tail":
                for q in range(N_CHUNK):
                    bad.add(q * 128 + int(p))
            else:
                bad.add(j * 128 + int(p))
    return bad


def _prep_host(u, y, ind, U, Y):
    """Scatter + device arrays (fp16) + fp64 base sum."""
    u = np.asarray(u, dtype=np.float32)
    y = np.asarray(y, dtype=np.float32)
    ind = np.asarray(ind).astype(np.int64)
    U2 = np.array(U, dtype=np.float32, copy=True)
    Y2 = np.array(Y, dtype=np.float32, copy=True)
    U2[ind] = u
    Y2[ind] = y

    u64 = u.astype(np.float64)
    U64 = U2.astype(np.float64)
    u_sq64 = (u64 * u64).sum(axis=1)            # [B]
    U_sq64 = (U64 * U64).sum(axis=1)            # [N]
    s_raw = (
        N * u_sq64.sum()
        + B * U_sq64.sum()
        - 2.0 * (u64.sum(axis=0) @ U64.sum(axis=0))
    )

    ypT = np.zeros((128, B), dtype=FP16)
    ypT[:C] = y.T.astype(FP16)
    ypT[C] = FP16(1.0)                          # bias row partner
    Yt = Y2.T                                   # [C, N]
    Ypk = np.zeros((128, N // 2), dtype=np.float32)
    Ypk[:C] = 0.5 * Yt[:, 0::2] + 128.0 * Yt[:, 1::2]
    Ypk = Ypk.astype(FP16)
    Ypk[C] = FP16(BIAS)

    return u, y, U2, Y2, ypT, Ypk, s_raw


def _full_numpy_loss(u, y, U2, Y2):
    """Exact fp64 fallback (blocked); only used if detector preconditions
    fail (non-binary labels) -- never on spec inputs."""
    total = 0.0
    U64 = U2.astype(np.float64)
    Y64 = Y2.astype(np.float64)
    U_sq = (U64 * U64).sum(axis=1)
    for b0 in range(0, B, 64):
        ub = u[b0 : b0 + 64].astype(np.float64)
        yb = y[b0 : b0 + 64].astype(np.float64)
        dist = np.maximum(
            (ub * ub).sum(1)[:, None] - 2.0 * (ub @ U64.T) + U_sq[None, :], 0.0)
        mism = (yb @ Y64.T) == 0.0
        total += np.where(mism, 0.5 * np.maximum(M_MARGIN - dist, 0.0),
                          0.5 * dist).sum()
    loss1 = total / (B * N)
    loss2 = ALPHA * np.abs(1.0 - np.sign(u)).mean(dtype=np.float64)
    return np.array(loss1 + loss2, dtype=np.float32)


def _detector_preconditions_ok(y, Y2):
    # binary labels -> the packed value and both bit/threshold tests are
    # exact (P spans 16 bits < fp32's 24-bit mantissa)
    return bool(((y == 0.0) | (y == 1.0)).all()
                and ((Y2 == 0.0) | (Y2 == 1.0)).all())


_PROG_CACHE = {}


def _get_program():
    key = ("v6", N_SH)
    if key not in _PROG_CACHE:
        _PROG_CACHE[key] = _build_program(N_SH)
    return _PROG_CACHE[key]


def kernel(u, y, ind, U, Y):
    u, y, U2, Y2, ypT, Ypk, s_raw = _prep_host(u, y, ind, U, Y)

    if not _detector_preconditions_ok(y, Y2):
        return _full_numpy_loss(u, y, U2, Y2)

    nc, tiles, ops, rem = _get_program()
    n_pk_sh = N_SH // 2
    in_maps = []
    for c in range(N_CORES):
        ns = slice(c * n_pk_sh, (c + 1) * n_pk_sh)
        in_maps.append({
            "ypT": ypT,
            "Ypk": _pad_shard(Ypk[:, ns]),
        })

    res = run_bass_kernel_spmd(nc, in_maps, list(range(N_CORES)))
    results = res.results

    bad_rows = set()
    for c in range(N_CORES):
        accA = np.asarray(results[c]["accA"], dtype=np.float64)
        accV = np.asarray(results[c]["accV"], dtype=np.float64)
        bad_rows |= _host_flags(accA, accV, tiles, ops)

    corr = 0.0
    if bad_rows:
        Y64 = Y2.astype(np.float64)
        U64 = U2.astype(np.float64)
        for b in sorted(bad_rows):
            match = y[b].astype(np.float64) @ Y64.T      # [N]
            zn = np.nonzero(match == 0.0)[0]
            if len(zn) == 0:
                continue
            d = u[b].astype(np.float64)[None, :] - U64[zn]
            raw = (d * d).sum(axis=1)
            corr += float(np.sum(np.maximum(M_MARGIN - raw, 0.0) - raw))

    total2 = s_raw + corr
    loss1 = 0.5 * total2 / (B * N)

    loss2 = ALPHA * np.abs(1.0 - np.sign(u)).mean(dtype=np.float64)

    return np.array(loss1 + loss2, dtype=np.float32)


# revision 18
# speedup vs baseline: 1.1776x; 1.1776x over previous
"""DSH loss kernel for Trainium2 (8 NeuronCores, Bass/Tile) — v4b.

Math (reference):
    U[ind] = u; Y[ind] = y
    raw[b,n]  = ||u_b||^2 - 2 u_b.U_n + ||U_n||^2          (>= 0 mathematically)
    dist      = max(raw, 0)
    match[b,n]= y_b . Y_n          (integer >= 0)
    m         = (match == 0)       ("mismatch" mask, statistically ~never 1)
    loss1 = mean( (1-m)*0.5*dist + m*0.5*relu(M - dist) )
    loss2 = ALPHA * mean(|1 - sign(u)|)

Decomposition:
    2*B*N*loss1 = S_raw + sum_{m=1} [ relu(M - raw) - raw ]
      S_raw factorizes: N*sum(u_sq) + B*sum(U_sq) - 2*colsum(u).colsum(U)
      -> computed exactly on host in fp64.
    The correction term needs the match==0 pairs. The ONLY thing the device
    must produce is a sound per-gallery-row flag "this row has a match==0
    pair"; the host recomputes flagged rows exactly in fp64. The distance
    matmul of earlier versions never changed the flag -> dropped entirely
    (halves PE work and gallery HBM traffic): one bf16 matmul per 128-row
    tile computes match directly (exact: {0,1} operands, fp32 PSUM).
    Detection is an exact integer zero-test, one elementwise pass per pair:
      - DVE groups:   col = min_b match     (tensor_reduce min; flag col<0.5)
      - ACT groups:   col = sum relu(.5-x)  (activation accum; flag col>0.25)
    The pass is the true bottleneck (every pair crosses ACT/DVE at
    1 elem/lane-cycle from fp32 PSUM; GpSimd has no PSUM port), so groups
    are greedily balanced across both engines by HW-measured cost
    (ACT 1401 ns/group incl read-accumulator, DVE 1220) and run
    concurrently on different PSUM buffers while PE fills ahead.

Device tiling per core (shard = 12500 U/Y rows, padded to 12544):
    98 tiles of 128 gallery rows; groups of 2 tiles share one [128,1024]
    PSUM tile (2 banks; 4 pool bufs = all 8 banks). Gallery DMA is sliced
    across the sync + gpsimd queues; the moving operand goes first on sync
    (the scalar queue starts ~3 us later behind engine init), and a dummy
    activation pulls the Relu table load off the first EW op's critical
    path.
"""

import numpy as np
import ml_dtypes

import concourse.bass as bass
import concourse.mybir as mybir
import concourse.tile as tile
from concourse import bacc
from concourse.bass_utils import run_bass_kernel_spmd

# Problem constants (hardcoded per harness contract)
B = 512
BIT = 64
C = 100
N = 100000
N_CORES = 8
N_SH = N // N_CORES          # 12500
M_MARGIN = 2.0 * BIT         # 128.0
ALPHA = 0.1
P_TILE = 128                 # gallery rows per tile (PSUM partition dim)
TILES_PER_GROUP = 2          # one [128, 1024] PSUM tile per group

# HW-measured per-group engine cost (ns) for greedy ACT/DVE balancing
_ACT_GROUP_NS = 1401.0       # ACTIVATE 1110 + READ_ACCUMULATOR 291
_DVE_GROUP_NS = 1220.0       # TENSOR_REDUCE

BF16 = ml_dtypes.bfloat16


def _schedule(n_groups: int):
    """Greedy engine assignment: 'A' (ScalarE relu-accum) or 'D' (VectorE
    min-reduce), balancing measured busy time."""
    eng = []
    t_act = t_dve = 0.0
    for _ in range(n_groups):
        if t_act + _ACT_GROUP_NS <= t_dve + _DVE_GROUP_NS:
            eng.append("A")
            t_act += _ACT_GROUP_NS
        else:
            eng.append("D")
            t_dve += _DVE_GROUP_NS
    return eng


def _build_program(n_sh: int):
    """v4b: match-only matmuls + exact integer zero-detection, dual-engine."""
    fp32 = mybir.dt.float32
    bf16 = mybir.dt.bfloat16
    nc = bacc.Bacc("TRN2", target_bir_lowering=False)

    g_cols = TILES_PER_GROUP * P_TILE                     # 256 gallery rows
    n_pad = ((n_sh + g_cols - 1) // g_cols) * g_cols
    n_tiles = n_pad // P_TILE
    n_groups = n_tiles // TILES_PER_GROUP
    engines = _schedule(n_groups)

    # K=128 zero-padded label operands ({0,1} values; rows C..127 zero)
    ypT_d = nc.declare_dram_parameter("ypT", [128, B], bf16, isOutput=False)
    YT_d = nc.declare_dram_parameter("YT", [128, n_sh], bf16, isOutput=False)
    accD_d = nc.declare_dram_parameter("accD", [128, n_groups], fp32, isOutput=True)

    with tile.TileContext(nc) as tc:
        with (
            tc.tile_pool(name="resident", bufs=1) as resident,
            tc.tile_pool(name="scr", bufs=2) as scrp,
            tc.tile_pool(name="psum", bufs=4, space="PSUM") as psump,
        ):
            yp_sb = resident.tile([128, B], bf16, tag="yp")
            YT_sb = resident.tile([128, n_pad], bf16, tag="YT")
            accD = resident.tile([128, n_groups], fp32, tag="accD")
            bias_h = resident.tile([128, 1], fp32, tag="biash")
            dscr = resident.tile([128, 1], bf16, tag="dscr")

            # Moving operand first on sync (needed by every matmul; the
            # scalar queue starts ~3us late behind engine init), then
            # gallery slices small-first, alternating sync/gpsimd queues.
            nc.sync.dma_start(yp_sb[:], ypT_d[:])
            nc.gpsimd.dma_start(YT_sb[:, :256], YT_d[:, :256])
            nc.sync.dma_start(YT_sb[:, 256:512], YT_d[:, 256:512])
            s = 512
            widths = [256, 256, 512, 512, 1024, 1024] + [2048] * 12
            qs = [nc.gpsimd, nc.sync]
            qi = 0
            for w in widths:
                if s >= n_sh:
                    break
                w = min(w, n_sh - s)
                qs[qi].dma_start(YT_sb[:, s : s + w], YT_d[:, s : s + w])
                qi ^= 1
                s += w
            if s < n_sh:
                half = (n_sh - s) // 2
                nc.sync.dma_start(YT_sb[:, s : s + half], YT_d[:, s : s + half])
                nc.gpsimd.dma_start(YT_sb[:, s + half : n_sh], YT_d[:, s + half :])

            # Pad COLUMNS of YT get 1.0, so a pad row matches any y_b with
            # at least one class; an all-zero y_b flags pad rows, which the
            # host skips (n_loc >= N_SH).
            if n_pad > n_sh:
                nc.vector.memset(YT_sb[:, n_sh:], 1.0)
            nc.vector.memset(bias_h[:], 0.5)
            nc.vector.memset(accD[:], 0.0)
            # dummy activation: pulls the ~1.3us Relu table load off the
            # first real EW op's critical path
            nc.scalar.activation(
                dscr[:], bias_h[:],
                mybir.ActivationFunctionType.Relu,
                bias=bias_h[:], scale=-1.0,
            )

            for g in range(n_groups):
                x = psump.tile([P_TILE, TILES_PER_GROUP * B], fp32, tag="x")
                for h in range(TILES_PER_GROUP):
                    t = TILES_PER_GROUP * g + h
                    ns = slice(t * P_TILE, (t + 1) * P_TILE)
                    nc.tensor.matmul(
                        x[:, h * B : (h + 1) * B],
                        lhsT=YT_sb[:, ns], rhs=yp_sb[:],
                        start=True, stop=True,
                    )

                col = accD[:, g : g + 1]
                if engines[g] == "A":
                    scr = scrp.tile([P_TILE, TILES_PER_GROUP * B], mybir.dt.bfloat16, tag="scrA")
                    # relu(0.5 - x): 0.5 per match==0 pair, else 0; col > 0.25
                    # iff some match==0 in this group
                    nc.scalar.activation(
                        scr[:], x[:],
                        mybir.ActivationFunctionType.Relu,
                        bias=bias_h[:], scale=-1.0,
                        accum_out=col,
                    )
                else:
                    # col = min over the group's 1024 match values; < 0.5 iff
                    # some match==0 here
                    nc.vector.tensor_reduce(
                        col, x[:], axis=mybir.AxisListType.X,
                        op=mybir.AluOpType.min,
                    )

            # split the output DMA: earlier chunks only depend on earlier
            # groups' cols, so they overlap the stream; only the last few
            # columns remain on the kernel tail
            c1 = max(0, n_groups - 8)
            c2 = max(0, n_groups - 3)
            if c1:
                nc.sync.dma_start(accD_d[:, :c1], accD[:, :c1])
            if c2 > c1:
                nc.sync.dma_start(accD_d[:, c1:c2], accD[:, c1:c2])
            nc.sync.dma_start(accD_d[:, c2:], accD[:, c2:])

    nc.finalize()
    return nc, n_groups, engines


def _prep_host(u, y, ind, U, Y):
    """Scatter + device arrays (bf16) + fp64 base sum."""
    u = np.asarray(u, dtype=np.float32)
    y = np.asarray(y, dtype=np.float32)
    ind = np.asarray(ind).astype(np.int64)
    U2 = np.array(U, dtype=np.float32, copy=True)
    Y2 = np.array(Y, dtype=np.float32, copy=True)
    U2[ind] = u
    Y2[ind] = y

    u64 = u.astype(np.float64)
    U64 = U2.astype(np.float64)
    u_sq64 = (u64 * u64).sum(axis=1)            # [B]
    U_sq64 = (U64 * U64).sum(axis=1)            # [N]
    s_raw = (
        N * u_sq64.sum()
        + B * U_sq64.sum()
        - 2.0 * (u64.sum(axis=0) @ U64.sum(axis=0))
    )

    # K=128 zero-padded label operands ({0,1} values exact in bf16)
    ypT = np.zeros((128, B), dtype=BF16)
    ypT[:C] = y.T.astype(BF16)
    YT = np.zeros((128, N), dtype=BF16)
    YT[:C] = Y2.T.astype(BF16)

    return u, y, U2, Y2, ypT, YT, s_raw


def _full_numpy_loss(u, y, U2, Y2):
    """Exact fp64 fallback (blocked); only used if detector preconditions
    fail (non-binary labels) -- never on spec inputs."""
    total = 0.0
    U64 = U2.astype(np.float64)
    Y64 = Y2.astype(np.float64)
    U_sq = (U64 * U64).sum(axis=1)
    for b0 in range(0, B, 64):
        ub = u[b0 : b0 + 64].astype(np.float64)
        yb = y[b0 : b0 + 64].astype(np.float64)
        dist = np.maximum(
            (ub * ub).sum(1)[:, None] - 2.0 * (ub @ U64.T) + U_sq[None, :], 0.0)
        mism = (yb @ Y64.T) == 0.0
        total += np.where(mism, 0.5 * np.maximum(M_MARGIN - dist, 0.0),
                          0.5 * dist).sum()
    loss1 = total / (B * N)
    loss2 = ALPHA * np.abs(1.0 - np.sign(u)).mean(dtype=np.float64)
    return np.array(loss1 + loss2, dtype=np.float32)


def _detector_preconditions_ok(y, Y2):
    # binary labels -> match is an exact small integer in bf16 matmul/fp32
    # PSUM, and the zero test is exact
    return bool(((y == 0.0) | (y == 1.0)).all()
                and ((Y2 == 0.0) | (Y2 == 1.0)).all())


_PROG_CACHE = {}


def _get_program():
    key = ("v4b", N_SH)
    if key not in _PROG_CACHE:
        _PROG_CACHE[key] = _build_program(N_SH)
    return _PROG_CACHE[key]


def kernel(u, y, ind, U, Y):
    u, y, U2, Y2, ypT, YT, s_raw = _prep_host(u, y, ind, U, Y)

    if not _detector_preconditions_ok(y, Y2):
        return _full_numpy_loss(u, y, U2, Y2)

    nc, n_groups, engines = _get_program()
    in_maps = []
    for c in range(N_CORES):
        ns = slice(c * N_SH, (c + 1) * N_SH)
        in_maps.append({
            "ypT": ypT,
            "YT": np.ascontiguousarray(YT[:, ns]),
        })

    res = run_bass_kernel_spmd(nc, in_maps, list(range(N_CORES)))
    results = res.results

    corr = 0.0
    for c in range(N_CORES):
        accD = np.asarray(results[c]["accD"], dtype=np.float64)
        flagged = set()
        for g, e in enumerate(engines):
            col = accD[:, g]
            ps = np.nonzero(col > 0.25)[0] if e == "A" else np.nonzero(col < 0.5)[0]
            for p in ps:
                flagged.add((int(p), g))
        for p, g in sorted(flagged):
            # group covers TILES_PER_GROUP tiles sharing partition p
            for h in range(TILES_PER_GROUP):
                n_loc = (TILES_PER_GROUP * g + h) * P_TILE + p
                if n_loc >= N_SH:
                    continue  # padded column
                n_glob = c * N_SH + n_loc
                match = y.astype(np.float64) @ Y2[n_glob].astype(np.float64)
                zrows = np.nonzero(match == 0.0)[0]
                for b in zrows:
                    d = u[b].astype(np.float64) - U2[n_glob].astype(np.float64)
                    raw = float(d @ d)
                    corr += max(M_MARGIN - raw, 0.0) - raw

    total2 = s_raw + corr
    loss1 = 0.5 * total2 / (B * N)

    sign_u = np.sign(u)
    loss2 = ALPHA * np.abs(1.0 - sign_u).mean(dtype=np.float64)

    return np.array(loss1 + loss2, dtype=np.float32)


# revision 19
# speedup vs baseline: 1.2153x; 1.0320x over previous
"""DSH loss kernel for Trainium2 (8 NeuronCores, Bass/Tile) — v10: v4 pipeline + packed batch pairs.

Math (reference):
    U[ind] = u; Y[ind] = y
    raw[b,n]  = ||u_b||^2 - 2 u_b.U_n + ||U_n||^2          (>= 0 mathematically)
    dist      = max(raw, 0)
    match[b,n]= y_b . Y_n          (integer >= 0)
    m         = (match == 0)       ("mismatch" mask, statistically ~never 1)
    loss1 = mean( (1-m)*0.5*dist + m*0.5*relu(M - dist) )
    loss2 = ALPHA * mean(|1 - sign(u)|)

Decomposition:
    2*B*N*loss1 = S_raw + sum_{m=1} [ relu(M - raw) - raw ]
      S_raw factorizes: N*sum(u_sq) + B*sum(U_sq) - 2*colsum(u).colsum(U)
      -> computed exactly on host in fp64.
    The correction term needs the match==0 pairs. The ONLY thing the device
    must produce is a sound per-gallery-row flag "this row has a match==0
    pair"; the host recomputes flagged rows exactly in fp64. The distance
    matmul of earlier versions never changed the flag -> dropped entirely
    (halves PE work and gallery HBM traffic): one bf16 matmul per 128-row
    tile computes match directly (exact: {0,1} operands, fp32 PSUM).
    Detection is an exact integer zero-test, one elementwise pass per pair:
      - DVE groups:   col = min_b match     (tensor_reduce min; flag col<0.5)
      - ACT groups:   col = sum relu(.5-x)  (activation accum; flag col>0.25)
    The pass is the true bottleneck (every pair crosses ACT/DVE at
    1 elem/lane-cycle from fp32 PSUM; GpSimd has no PSUM port), so groups
    are greedily balanced across both engines by HW-measured cost
    (ACT 1401 ns/group incl read-accumulator, DVE 1220) and run
    concurrently on different PSUM buffers while PE fills ahead.

Device tiling per core (shard = 12500 U/Y rows, padded to 12544):
    98 tiles of 128 gallery rows; groups of 2 tiles share one [128,1024]
    PSUM tile (2 banks; 4 pool bufs = all 8 banks). Gallery DMA is sliced
    across the sync + gpsimd queues; the moving operand goes first on sync
    (the scalar queue starts ~3 us later behind engine init), and a dummy
    activation pulls the Relu table load off the first EW op's critical
    path.
"""

import numpy as np
import ml_dtypes

import concourse.bass as bass
import concourse.mybir as mybir
import concourse.tile as tile
from concourse import bacc
from concourse.bass_utils import run_bass_kernel_spmd

# Problem constants (hardcoded per harness contract)
B = 512
BIT = 64
C = 100
N = 100000
N_CORES = 8
N_SH = N // N_CORES          # 12500
M_MARGIN = 2.0 * BIT         # 128.0
ALPHA = 0.1
P_TILE = 128                 # gallery rows per tile (PSUM partition dim)
TILES_PER_GROUP = 4          # one [128, 1024] PSUM tile per group (4x256)
B_PK = 256                   # packed batch cols (pairs, radix 128)
BIAS = 16384.0               # exponent-pinning bias (2^14)
OTHR = 16498.0               # O-threshold: 16484 < t < 16512
EEPS = float(2.0 ** -124)    # E bias: relu(EEPS - L) fires only at L <= +0

# HW-measured per-group engine cost (ns) for greedy ACT/DVE balancing
_ACT_GROUP_NS = 1401.0       # ACTIVATE 1110 + READ_ACCUMULATOR 291
_DVE_GROUP_NS = 1220.0       # TENSOR_REDUCE

BF16 = ml_dtypes.bfloat16


def _schedule(n_groups: int):
    """Per group, two tests (O: odd-member zero via threshold on P,
    E: even-member zero via low-bf16-half-of-P == +0), each greedily
    assigned to ScalarE ('A') or VectorE ('D') by measured busy time."""
    ops = []
    t_act = t_dve = 0.0
    col = 0
    for _ in range(n_groups):
        pair = []
        for test in ("O", "E"):
            if t_act + _ACT_GROUP_NS <= t_dve + _DVE_GROUP_NS:
                pair.append((test, "A", col))
                t_act += _ACT_GROUP_NS
            else:
                pair.append((test, "D", col))
                t_dve += _DVE_GROUP_NS
            col += 1
        ops.append(pair)
    return ops


def _build_program(n_sh: int):
    """v4b: match-only matmuls + exact integer zero-detection, dual-engine."""
    fp32 = mybir.dt.float32
    bf16 = mybir.dt.bfloat16
    nc = bacc.Bacc("TRN2", target_bir_lowering=False)

    g_cols = TILES_PER_GROUP * P_TILE                     # 512 gallery rows
    n_pad = ((n_sh + g_cols - 1) // g_cols) * g_cols
    n_tiles = n_pad // P_TILE
    n_groups = n_tiles // TILES_PER_GROUP
    ops = _schedule(n_groups)

    # moving operand: batch pairs packed radix-128 + 16384 bias row;
    # stationary: gallery labels with row C == 1.0 (bias partner)
    ypT_d = nc.declare_dram_parameter("ypT", [128, B_PK], bf16, isOutput=False)
    YT_d = nc.declare_dram_parameter("YT", [128, n_sh], bf16, isOutput=False)
    accD_d = nc.declare_dram_parameter("accD", [128, 2 * n_groups], fp32, isOutput=True)

    with tile.TileContext(nc) as tc:
        with (
            tc.tile_pool(name="resident", bufs=1) as resident,
            tc.tile_pool(name="scr", bufs=2) as scrp,
            tc.tile_pool(name="psum", bufs=4, space="PSUM") as psump,
        ):
            yp_sb = resident.tile([128, B_PK], bf16, tag="yp")
            YT_sb = resident.tile([128, n_pad], bf16, tag="YT")
            accD = resident.tile([128, 2 * n_groups], fp32, tag="accD")
            bias_o = resident.tile([128, 1], fp32, tag="biaso")
            bias_e = resident.tile([128, 1], fp32, tag="biase")
            dscr = resident.tile([128, 1], bf16, tag="dscr")

            # Moving operand first on sync (needed by every matmul; the
            # scalar queue starts ~3us late behind engine init), then
            # gallery slices small-first, alternating sync/gpsimd queues.
            nc.sync.dma_start(yp_sb[:], ypT_d[:])
            nc.gpsimd.dma_start(YT_sb[:, :256], YT_d[:, :256])
            nc.sync.dma_start(YT_sb[:, 256:512], YT_d[:, 256:512])
            s = 512
            widths = [256, 256, 512, 512, 1024, 1024] + [2048] * 12
            qs = [nc.gpsimd, nc.sync]
            qi = 0
            for w in widths:
                if s >= n_sh:
                    break
                w = min(w, n_sh - s)
                qs[qi].dma_start(YT_sb[:, s : s + w], YT_d[:, s : s + w])
                qi ^= 1
                s += w
            if s < n_sh:
                half = (n_sh - s) // 2
                nc.sync.dma_start(YT_sb[:, s : s + half], YT_d[:, s : s + half])
                nc.gpsimd.dma_start(YT_sb[:, s + half : n_sh], YT_d[:, s + half :])

            # Pad COLUMNS of YT get 1.0, so a pad row matches any y_b with
            # at least one class; an all-zero y_b flags pad rows, which the
            # host skips (n_loc >= N_SH).
            if n_pad > n_sh:
                nc.vector.memset(YT_sb[:, n_sh:], 1.0)
            nc.vector.memset(bias_o[:], OTHR)
            nc.vector.memset(bias_e[:], EEPS)
            nc.vector.memset(accD[:], 0.0)
            # dummy activation: pulls the ~1.3us Relu table load off the
            # first real EW op's critical path
            nc.scalar.activation(
                dscr[:], bias_o[:],
                mybir.ActivationFunctionType.Relu,
                bias=bias_e[:], scale=-1.0,
            )

            gw = TILES_PER_GROUP * B_PK                   # 1024 fp32 cols
            for g in range(n_groups):
                x = psump.tile([P_TILE, gw], fp32, tag="x")
                for h in range(TILES_PER_GROUP):
                    t = TILES_PER_GROUP * g + h
                    ns = slice(t * P_TILE, (t + 1) * P_TILE)
                    nc.tensor.matmul(
                        x[:, h * B_PK : (h + 1) * B_PK],
                        lhsT=YT_sb[:, ns], rhs=yp_sb[:],
                        start=True, stop=True,
                    )

                xb = x[:].bitcast(bf16)
                xlow = xb[:, 0::2]
                for test, eng, col in sorted(ops[g], key=lambda o: o[1] != "D"):
                    acol = accD[:, col : col + 1]
                    if eng == "D":
                        if test == "O":
                            scr = scrp.tile([P_TILE, gw], mybir.dt.bfloat16, tag="sDO")
                            # min(P-16498, 0) min-accum: <= -14 iff m_o==0
                            nc.vector.tensor_scalar(
                                scr[:], x[:], OTHR, 0.0,
                                mybir.AluOpType.subtract, mybir.AluOpType.min,
                                accum_out=acol,
                            )
                        else:
                            # min over low bf16 halves: +0 iff some m_e==0
                            nc.vector.tensor_reduce(
                                acol, xlow, axis=mybir.AxisListType.X,
                                op=mybir.AluOpType.min,
                            )
                    else:
                        scr = scrp.tile([P_TILE, gw], mybir.dt.bfloat16,
                                        tag="sA" + test)
                        if test == "O":
                            # relu(16498 - P): >= 14 per m_o==0 pair
                            nc.scalar.activation(
                                scr[:], x[:],
                                mybir.ActivationFunctionType.Relu,
                                bias=bias_o[:], scale=-1.0,
                                accum_out=acol,
                            )
                        else:
                            # relu(2^-124 - L): fires only at L <= +0
                            nc.scalar.activation(
                                scr[:], xlow,
                                mybir.ActivationFunctionType.Relu,
                                bias=bias_e[:], scale=-1.0,
                                accum_out=acol,
                            )

            # split the output DMA: earlier chunks only depend on earlier
            # groups' cols, so they overlap the stream; only the last few
            # columns remain on the kernel tail
            c1 = max(0, 2 * n_groups - 8)
            c2 = max(0, 2 * n_groups - 3)
            if c1:
                nc.sync.dma_start(accD_d[:, :c1], accD[:, :c1])
            if c2 > c1:
                nc.sync.dma_start(accD_d[:, c1:c2], accD[:, c1:c2])
            nc.sync.dma_start(accD_d[:, c2:], accD[:, c2:])

    nc.finalize()
    return nc, n_groups, ops


def _prep_host(u, y, ind, U, Y):
    """Scatter + device arrays (bf16) + fp64 base sum."""
    u = np.asarray(u, dtype=np.float32)
    y = np.asarray(y, dtype=np.float32)
    ind = np.asarray(ind).astype(np.int64)
    U2 = np.array(U, dtype=np.float32, copy=True)
    Y2 = np.array(Y, dtype=np.float32, copy=True)
    U2[ind] = u
    Y2[ind] = y

    u64 = u.astype(np.float64)
    U64 = U2.astype(np.float64)
    u_sq64 = (u64 * u64).sum(axis=1)            # [B]
    U_sq64 = (U64 * U64).sum(axis=1)            # [N]
    s_raw = (
        N * u_sq64.sum()
        + B * U_sq64.sum()
        - 2.0 * (u64.sum(axis=0) @ U64.sum(axis=0))
    )

    # moving: batch pairs packed radix-128 ({0,1,128,129} bf16-exact)
    # plus the 16384 bias row; stationary: labels with row C == 1.0
    yt = y.T.astype(np.float32)
    ypT = np.zeros((128, B_PK), dtype=np.float32)
    ypT[:C] = yt[:, 0::2] + 128.0 * yt[:, 1::2]
    ypT = ypT.astype(BF16)
    ypT[C] = BF16(BIAS)
    YT = np.zeros((128, N), dtype=BF16)
    YT[:C] = Y2.T.astype(BF16)
    YT[C] = BF16(1.0)

    return u, y, U2, Y2, ypT, YT, s_raw


def _full_numpy_loss(u, y, U2, Y2):
    """Exact fp64 fallback (blocked); only used if detector preconditions
    fail (non-binary labels) -- never on spec inputs."""
    total = 0.0
    U64 = U2.astype(np.float64)
    Y64 = Y2.astype(np.float64)
    U_sq = (U64 * U64).sum(axis=1)
    for b0 in range(0, B, 64):
        ub = u[b0 : b0 + 64].astype(np.float64)
        yb = y[b0 : b0 + 64].astype(np.float64)
        dist = np.maximum(
            (ub * ub).sum(1)[:, None] - 2.0 * (ub @ U64.T) + U_sq[None, :], 0.0)
        mism = (yb @ Y64.T) == 0.0
        total += np.where(mism, 0.5 * np.maximum(M_MARGIN - dist, 0.0),
                          0.5 * dist).sum()
    loss1 = total / (B * N)
    loss2 = ALPHA * np.abs(1.0 - np.sign(u)).mean(dtype=np.float64)
    return np.array(loss1 + loss2, dtype=np.float32)


def _detector_preconditions_ok(y, Y2):
    # binary labels -> match is an exact small integer in bf16 matmul/fp32
    # PSUM, and the zero test is exact
    return bool(((y == 0.0) | (y == 1.0)).all()
                and ((Y2 == 0.0) | (Y2 == 1.0)).all())


_PROG_CACHE = {}


def _get_program():
    key = ("v10", N_SH)
    if key not in _PROG_CACHE:
        _PROG_CACHE[key] = _build_program(N_SH)
    return _PROG_CACHE[key]


def kernel(u, y, ind, U, Y):
    u, y, U2, Y2, ypT, YT, s_raw = _prep_host(u, y, ind, U, Y)

    if not _detector_preconditions_ok(y, Y2):
        return _full_numpy_loss(u, y, U2, Y2)

    nc, n_groups, ops = _get_program()
    in_maps = []
    for c in range(N_CORES):
        ns = slice(c * N_SH, (c + 1) * N_SH)
        in_maps.append({
            "ypT": ypT,
            "YT": np.ascontiguousarray(YT[:, ns]),
        })

    res = run_bass_kernel_spmd(nc, in_maps, list(range(N_CORES)))
    results = res.results

    corr = 0.0
    for c in range(N_CORES):
        accD = np.asarray(results[c]["accD"], dtype=np.float64)
        flagged = set()
        for g in range(n_groups):
            hit = np.zeros(128, bool)
            for test, e, col in ops[g]:
                cv = accD[:, col]
                if e == "A":
                    hit |= cv > (7.0 if test == "O" else 2.0 ** -125)
                else:
                    hit |= cv < (-7.0 if test == "O" else 1.17e-38)
            for p in np.nonzero(hit)[0]:
                flagged.add((int(p), g))
        for p, g in sorted(flagged):
            # group covers TILES_PER_GROUP tiles sharing partition p
            for h in range(TILES_PER_GROUP):
                n_loc = (TILES_PER_GROUP * g + h) * P_TILE + p
                if n_loc >= N_SH:
                    continue  # padded column
                n_glob = c * N_SH + n_loc
                match = y.astype(np.float64) @ Y2[n_glob].astype(np.float64)
                zrows = np.nonzero(match == 0.0)[0]
                for b in zrows:
                    d = u[b].astype(np.float64) - U2[n_glob].astype(np.float64)
                    raw = float(d @ d)
                    corr += max(M_MARGIN - raw, 0.0) - raw

    total2 = s_raw + corr
    loss1 = 0.5 * total2 / (B * N)

    sign_u = np.sign(u)
    loss2 = ALPHA * np.abs(1.0 - sign_u).mean(dtype=np.float64)

    return np.array(loss1 + loss2, dtype=np.float32)
